# revision 34
# baseline (speedup 1.0000x reference)
"""Trainium2 Bass kernel for nn_RahmanDynamicNet:
conv(1->20,(34,5)) -> BN(eval) -> sigmoid -> ParametricLIF -> linear(20->1)
-> sigmoid -> ParametricLIF -> [B,T] float32.

Self-contained: takes FULL inputs, shards T across 8 NeuronCores (SPMD, no
collectives), returns the FULL [B,T] output.

Math (same identities as the tuned v1 baseline):
  - Conv output feeds sigmoid => y in (0,1); LIF state v stays << VTH=1000,
    so spikes never fire and both LIF layers are pure EMAs ->
    tensor_tensor_scan (no T-loop).
  - EMA commutes with the linear readout: lin(EMA(u)) = EMA(lin(u)).
  - T sharded with a 25-step EMA warmup per core (state error ~0.5^25);
    core 0 is exact (scan decay zeroed at the warmup boundary column).

Perf structure (HW-measured findings, vs the v1 baseline at ~32us/rep):
  - S=25 outputs per block: each block's 500 conv columns exactly fill one
    PSUM bank, so the sigmoid is a single-bank contiguous ACT read with
    bf16 out (multi-bank bf16 ACT reads silently corrupt); the 172-cycle
    PSUM bubble amortizes over 500 cols (v1: 480/3-block groups).
  - matmul start=True clears has_written for the WHOLE psum bank, so each
    block issues ONE full-width start (chunk 0, which also carries every
    column's BN-shift bias on its j=2 ones row) + 9 band accumulates.
  - lhsT patch rows (j,ch) have uniform stride B in the [t, ch, b] fp8
    layout, so each 105-row chunk loads with ONE windowed DMA
    [[B,105],[25*st,21],[1,B]]. Measured: this AP shape sustains
    ~320-360 GB/s, while wide flat APs (small middle dim) collapse to
    ~25 GB/s. 10 load DMAs + 3 store DMAs per rep (v1: 44).
  - WARM=25, TL=525 (vs 576): 9% less of everything.
  - h-contraction: bf16 tensor_mul by tiled lin_w*sw1 + pairwise add tree
    20->10->5; the 10->5 level runs on the idle GPSIMD, the rest plus the
    EMA scans on DVE (2x bf16 perf mode), z-sigmoid on ACT.
  - No output copy: v EMA is stored unscaled, DMA'd straight out; the
    final *sw2 scale happens on host after the gather.
"""
import numpy as np
from contextlib import ExitStack
import sys

sys.path.insert(0, "/opt/trn_rl_repo")

import concourse.bass as bass
import concourse.bacc as bacc
import concourse.tile as tile
from concourse import mybir
from concourse.bass_utils import run_bass_kernel_spmd
import ml_dtypes

BF16 = ml_dtypes.bfloat16

B, F, T, H, K = 128, 34, 4000, 20, 5
NCORES = 8
S = 25           # outputs per block (500 cols = one PSUM bank)
JW = S + 4       # patch window (taps)
FA = F + 1       # augmented channels (x + ones)
ROWS = JW * FA   # 1015
NCHUNK = 10
CHROWS = 105     # 10*105 = 1050 (35 zero-pad rows)
NB = 21          # blocks per core
UGRP = 7         # blocks per h-contraction group
NG = NB // UGRP  # 3 groups
TL = NB * S      # 525
WARM = 25
TO = T // NCORES     # 500
PADL = 48
XT_W = 530
BCOLS = S * H        # 500
GT_T = UGRP * S      # 175 t per group
BN_EPS = 1e-5

_DT = mybir.dt

# (chunk, (col0, col1)) — per-block matmul list. start=True clears
# has_written for the WHOLE psum bank, so chunk 0 is the single start:
# full width, with every column's BN-shift bias on its ones row; chunks
# 1..9 accumulate their x-band column ranges.
MMLIST = [
    (0, (0, 500)),
    (1, (0, 120)),
    (2, (40, 180)),
    (3, (100, 240)),
    (4, (160, 300)),
    (5, (220, 360)),
    (6, (280, 420)),
    (7, (340, 480)),
    (8, (400, 500)),
    (9, (460, 500)),
]


def _sigmoid(v):
    return 1.0 / (1.0 + np.exp(-v))


def build_nc(sw1, sw2, reps=1):
    nc = bacc.Bacc()
    xt = nc.declare_dram_parameter("xt", [XT_W, FA, B], _DT.float8e3,
                                   isOutput=False)
    wcp = nc.declare_dram_parameter("wc", [CHROWS, NCHUNK, BCOLS], _DT.bfloat16,
                                    isOutput=False)
    wrp = nc.declare_dram_parameter("wrep", [B, UGRP * BCOLS], _DT.bfloat16,
                                    isOutput=False)
    cst = nc.declare_dram_parameter("consts", [B, 4], _DT.float32, isOutput=False)
    d0ap = nc.declare_dram_parameter("d0a", [B, TL], _DT.float32, isOutput=False)
    d0bp = nc.declare_dram_parameter("d0b", [B, TL], _DT.float32, isOutput=False)
    outp = nc.declare_dram_parameter("out", [B, TO], _DT.float32, isOutput=True)

    st_t = FA * B  # xt t-stride in elements

    with ExitStack() as ctx:
        tc = ctx.enter_context(tile.TileContext(nc))
        singles = ctx.enter_context(tc.tile_pool(name="singles", bufs=1))
        xp = ctx.enter_context(tc.tile_pool(name="xp", bufs=3))
        pp = ctx.enter_context(tc.tile_pool(name="pp", bufs=8, space="PSUM"))
        up = ctx.enter_context(tc.tile_pool(name="up", bufs=3))
        ump = ctx.enter_context(tc.tile_pool(name="ump", bufs=2))
        um2p = ctx.enter_context(tc.tile_pool(name="um2p", bufs=2))
        um3p = ctx.enter_context(tc.tile_pool(name="um3p", bufs=3))

        wc_sb = singles.tile([CHROWS, NCHUNK, BCOLS], _DT.bfloat16)
        nc.sync.dma_start(out=wc_sb, in_=wcp[:, :, :])
        wrep_sb = singles.tile([B, UGRP * BCOLS], _DT.bfloat16)
        nc.sync.dma_start(out=wrep_sb, in_=wrp[:, :])
        cst_sb = singles.tile([B, 4], _DT.float32)
        nc.sync.dma_start(out=cst_sb, in_=cst[:, :])
        d0a_sb = singles.tile([B, TL], _DT.float32)
        nc.sync.dma_start(out=d0a_sb, in_=d0ap[:, :])
        d0b_sb = singles.tile([B, TL], _DT.float32)
        nc.sync.dma_start(out=d0b_sb, in_=d0bp[:, :])

        pqzv = ctx.enter_context(tc.tile_pool(name="pqzv", bufs=2))

        xt_ap = xt[:, :, :]

        pend = None  # previous rep's (q, z, v) tail, emitted late so the
        # z-sigmoid / out-DMA never sit in the ACT/SP HWDGE FIFOs ahead of
        # the next rep's chunk-load DMAs (which would stall PE on the tail).
        for _rep in range(reps):
            p_sb = pqzv.tile([B, TL], _DT.float32)
            q_sb = pqzv.tile([B, TL], _DT.float32)
            z_sb = pqzv.tile([B, TL], _DT.float32)
            v_sb = pqzv.tile([B, TL], _DT.float32)
            # one windowed DMA per 105-row chunk loads the whole rep's
            # lhsT patches: [105 rows, 21 blocks, 128 b] (~320 GB/s shape).
            um3s = []
            xb = xp.tile([CHROWS, NCHUNK, NB, B], _DT.float8e3)
            for c in range(NCHUNK):
                src = bass.AP(
                    tensor=xt_ap.tensor,
                    offset=c * CHROWS * B,
                    ap=[[B, CHROWS], [S * st_t, NB], [1, B]],
                )
                eng = nc.sync if (c % 3 != 2) else nc.scalar
                eng.dma_start(out=xb[:, c, :, :], in_=src)
            if pend is not None:
                _emit_tail(nc, pend, d0b_sb, cst_sb, outp)
                pend = None

            for g in range(NG):
                u_g = up.tile([B, UGRP * BCOLS], _DT.bfloat16)
                for e in range(UGRP):
                    ibl = UGRP * g + e
                    psb = pp.tile([B, 512], _DT.float32)
                    for mi, (c, (a0, a1)) in enumerate(MMLIST):
                        nc.tensor.matmul(
                            psb[:, a0:a1], xb[:, c, ibl, :],
                            wc_sb[:, c, a0:a1],
                            start=(mi == 0), stop=(mi == len(MMLIST) - 1),
                            skip_group_check=True,
                        )
                    nc.scalar.activation(
                        out=u_g[:, BCOLS * e:BCOLS * (e + 1)],
                        in_=psb[:, 0:BCOLS],
                        func=mybir.ActivationFunctionType.Sigmoid,
                    )

                # h-contraction: p[:, t] = sum_h u*wrep; mul + 20->10 add
                # on DVE (2x bf16; middle group's add on Pool to balance),
                # 10->5 on Pool, 5->1 reduce on DVE.
                um = ump.tile([B, UGRP * BCOLS], _DT.bfloat16)
                nc.vector.tensor_mul(um[:, :], u_g[:, :], wrep_sb[:, :])
                umv = um.rearrange("p (t h) -> p t h", h=H)
                um2 = um2p.tile([B, GT_T * 10], _DT.bfloat16)
                um2v = um2.rearrange("p (t h) -> p t h", h=10)
                l1eng = nc.gpsimd if g == 1 else nc.vector
                l1eng.tensor_add(um2v[:, :, :], umv[:, :, 0:10],
                                 umv[:, :, 10:20])
                um3 = um3p.tile([B, GT_T * 5], _DT.bfloat16)
                um3v = um3.rearrange("p (t h) -> p t h", h=5)
                nc.gpsimd.tensor_add(um3v[:, :, :], um2v[:, :, 0:5],
                                     um2v[:, :, 5:10])
                um3s.append(um3v)

                # the reduce + q-scan of group g-1 are emitted HERE (one
                # group late) so the reduce never sits at the DVE queue
                # head waiting on Pool's 10->5 add — the next group's
                # mul/L1 run in the meantime. The z/v/out tail runs once
                # per rep and overlaps the next rep via the pooled buffers.
                if g >= 1:
                    _emit_red_q(nc, g - 1, um3s[g - 1], p_sb, q_sb, d0a_sb)
            _emit_red_q(nc, NG - 1, um3s[NG - 1], p_sb, q_sb, d0a_sb)
            pend = (q_sb, z_sb, v_sb)
        _emit_tail(nc, pend, d0b_sb, cst_sb, outp)
    nc.compile()
    return nc


def _emit_tail(nc, pend, d0b_sb, cst_sb, outp):
    q_sb, z_sb, v_sb = pend
    nc.scalar.activation(
        out=z_sb[:, :], in_=q_sb[:, :],
        func=mybir.ActivationFunctionType.Sigmoid,
        bias=cst_sb[:, 2:3],
    )
    nc.vector.tensor_tensor_scan(
        out=v_sb[:, :], data0=d0b_sb[:, :], data1=z_sb[:, :],
        initial=0.0,
        op0=mybir.AluOpType.mult, op1=mybir.AluOpType.add,
    )
    nc.sync.dma_start(out=outp[:, :], in_=v_sb[:, WARM:WARM + TO])


def _emit_red_q(nc, g, um3v, p_sb, q_sb, d0a_sb):
    s0, s1 = GT_T * g, GT_T * (g + 1)
    nc.vector.tensor_reduce(
        out=p_sb[:, s0:s1], in_=um3v[:, :, :],
        axis=mybir.AxisListType.X, op=mybir.AluOpType.add,
    )
    nc.vector.tensor_tensor_scan(
        out=q_sb[:, s0:s1], data0=d0a_sb[:, s0:s1], data1=p_sb[:, s0:s1],
        initial=(0.0 if g == 0 else q_sb[:, s0 - 1:s0]),
        op0=mybir.AluOpType.mult, op1=mybir.AluOpType.add,
    )


def prep(x, conv_w, conv_b, bn_gamma, bn_beta, bn_mean, bn_var,
         lin_w, lin_b, w1, w2):
    x = np.asarray(x, np.float32)
    inv = (np.asarray(bn_gamma, np.float32)
           / np.sqrt(np.asarray(bn_var, np.float32) + BN_EPS))
    shift = (np.asarray(conv_b, np.float32)
             - np.asarray(bn_mean, np.float32)) * inv \
        + np.asarray(bn_beta, np.float32)
    sw1 = float(_sigmoid(np.float32(np.asarray(w1))))
    sw2 = float(_sigmoid(np.float32(np.asarray(w2))))
    linb = float(np.asarray(lin_b, np.float32).reshape(-1)[0])
    lw = np.asarray(lin_w, np.float32).reshape(-1)

    GT = PADL + T + 40
    x_aug = np.zeros((GT, FA, B), np.float32)
    x_aug[PADL:PADL + T, :F, :] = x[:, 0].transpose(2, 1, 0)
    x_aug[PADL:PADL + T, F, :] = 1.0
    x_aug_f8 = x_aug.astype(ml_dtypes.float8_e3m4)

    cw = np.asarray(conv_w, np.float32)[:, 0]  # [H,F,K]
    Wf = np.zeros((NCHUNK * CHROWS, BCOLS), np.float32)
    for i in range(S):
        for k in range(K):
            j = i + k
            Wf[j * FA:j * FA + F, i * H:(i + 1) * H] = \
                (cw[:, :, k] * inv[:, None]).T
        # all BN-shift biases on chunk-0's j=2 ones row (row 104): keeps the
        # single full-width start=True on chunk 0, and t=g0+25*ibl+2 stays
        # inside the real ones region for every core/block that matters.
        Wf[2 * FA + F, i * H:(i + 1) * H] = shift
    wc = np.ascontiguousarray(
        Wf.reshape(NCHUNK, CHROWS, BCOLS).transpose(1, 0, 2)).astype(BF16)

    wr = np.tile(lw * sw1, UGRP * S).astype(BF16)
    wrep = np.ascontiguousarray(np.broadcast_to(wr, (B, UGRP * BCOLS)))

    consts = np.zeros((B, 4), np.float32)
    consts[:, 0] = 1.0 - sw1
    consts[:, 1] = 1.0 - sw2
    consts[:, 2] = linb

    d0a = np.full((B, TL), 1.0 - sw1, np.float32)
    d0b = np.full((B, TL), 1.0 - sw2, np.float32)
    d0a0 = d0a.copy(); d0a0[:, WARM] = 0.0
    d0b0 = d0b.copy(); d0b0[:, WARM] = 0.0

    in_maps = []
    for c in range(NCORES):
        g0 = 500 * c + PADL - WARM - 2
        xtc = np.ascontiguousarray(x_aug_f8[g0:g0 + XT_W, :, :])
        in_maps.append({"xt": xtc, "wc": wc, "wrep": wrep, "consts": consts,
                        "d0a": d0a0 if c == 0 else d0a,
                        "d0b": d0b0 if c == 0 else d0b})
    return in_maps, sw1, sw2


_NC_CACHE = {}


def kernel(**inputs):
    in_maps, sw1, sw2 = prep(**inputs)
    key = (round(sw1, 9), round(sw2, 9))
    if key not in _NC_CACHE:
        _NC_CACHE[key] = build_nc(sw1, sw2)
    nc = _NC_CACHE[key]
    res = run_bass_kernel_spmd(nc, in_maps, list(range(NCORES)))
    outs = [np.asarray(res.results[c]["out"], np.float32)
            for c in range(NCORES)]
    return np.float32(sw2) * np.concatenate(outs, axis=1)
